# revision 11
# baseline (speedup 1.0000x reference)
"""Causal self-attention (B=2, T=2048, D=1024, H=16) on 8 trn2 cores.

Sharding: tensor-parallel over heads x data-parallel over batch.
Core c handles batch b = c // 4, head group g = c % 4 (heads 4g..4g+3).
Host pre-slices/pre-transposes weight+activation shards (cast to bf16);
each core returns a partial y (its heads' contribution); host sums
groups of 4.

v2: bf16 matmul datapath (f32r runs fp32_mode=HIGH at ~2x the cycle
cost and kept tripping the PE clock throttle), causal trimming of the
diagonal S/PV quads via partial-width matmuls, and a tt-major pipeline
that normalizes + projects each 512-column t-tile as soon as its
attention finishes (no serial tail).
"""

import os
import sys

for _p in ("/opt/trn_rl_repo", "/root/.axon_site/_ro/trn_rl_repo"):
    if os.path.isdir(_p) and _p not in sys.path:
        sys.path.insert(0, _p)

import ml_dtypes
import numpy as np

import concourse.bass as bass
import concourse.mybir as mybir
import concourse.tile as tile
from concourse import bacc
from concourse.bass_utils import run_bass_kernel_spmd

F32 = mybir.dt.float32
BF16 = mybir.dt.bfloat16
U16 = mybir.dt.uint16

B, T, C = 2, 2048, 1024
NHEAD_TOT = 16
DH = 64
NCORES = 8
NH = 4          # heads per core
NPAIR = 2       # head pairs per core
CK = C // 128   # contraction chunks (8)
TT = 512        # attention t-tile width
NTT = T // TT   # 4
NSCH = T // 128  # s chunks (16)
FQK = 2 * NH * DH  # 512 cols of qkv^T for q+k
FV = NH * DH       # 256 cols for v

ONE_BF16 = 0x3F80


def build_nc():
    nc = bacc.Bacc("TRN2", target_bir_lowering=False, debug=False)

    xT = nc.dram_tensor("xT", [C, T], BF16, kind="ExternalInput")
    wqkvT = nc.dram_tensor("wqkvT", [C, FQK + FV], BF16, kind="ExternalInput")
    woutT = nc.dram_tensor("woutT", [NH * DH, C], BF16, kind="ExternalInput")
    y = nc.dram_tensor("y", [T, C], F32, kind="ExternalOutput")

    EXP = mybir.ActivationFunctionType.Exp

    with tile.TileContext(nc) as tc:
        with (
            tc.tile_pool(name="const", bufs=1) as const,
            tc.tile_pool(name="ptp", bufs=4) as ptp,
            tc.tile_pool(name="rcp", bufs=2) as rcp,
            tc.tile_pool(name="yp", bufs=2) as yp,
            tc.tile_pool(name="psA", bufs=2, space="PSUM") as psA,
            tc.tile_pool(name="psV", bufs=3, space="PSUM") as psV,
            tc.tile_pool(name="psB", bufs=1, space="PSUM") as psB,
        ):
            # ---- persistent SBUF ----
            xT_sb = const.tile([128, CK, T], BF16)            # x^T (c-major)
            wqkvT_sb = const.tile([128, CK, FQK + FV], BF16)  # W_qkv^T cols [q|k|v]
            woutT_sb = const.tile([128, NPAIR, C], BF16)      # W_out^T rows per pair
            qkT_sb = const.tile([128, 4, T], BF16)            # [qP0|qP1|kP0|kP1] x T
            v_sb = const.tile([128, NSCH, NH, DH + 1], BF16)  # V (s-major) + ones col
            oT_sb = const.tile([128, NPAIR, T], BF16)         # O^T, pair-stacked

            for ci in range(CK):
                nc.sync.dma_start(xT_sb[:, ci, :], xT[ci * 128:(ci + 1) * 128, :])
                nc.sync.dma_start(wqkvT_sb[:, ci, :], wqkvT[ci * 128:(ci + 1) * 128, :])
            for pr in range(NPAIR):
                nc.sync.dma_start(woutT_sb[:, pr, :], woutT[pr * 128:(pr + 1) * 128, :])
            nc.vector.memset(v_sb[:, :, :, DH:DH + 1].bitcast(U16), ONE_BF16)
            # per-half selector rows for the 1/L row->partition broadcast:
            # selh[0] = [1]*64+[0]*64, selh[1] = [0]*64+[1]*64
            selh = const.tile([1, 2, 128], BF16)
            nc.vector.memset(selh.bitcast(U16), 0)
            nc.vector.memset(selh[0:1, 0, 0:64].bitcast(U16), ONE_BF16)
            nc.vector.memset(selh[0:1, 1, 64:128].bitcast(U16), ONE_BF16)

            # ---- QKV projection ----
            # q^T/k^T: psum[f128, t512] = sum_c wqkvT[c, f].T @ xT[c, t]
            for ft in range(4):
                for tt in range(NTT):
                    ps = psA.tile([128, 2, TT], F32)
                    for ci in range(CK):
                        nc.tensor.matmul(
                            ps[:, 0, :],
                            wqkvT_sb[:, ci, ft * 128:(ft + 1) * 128],
                            xT_sb[:, ci, tt * TT:(tt + 1) * TT],
                            start=(ci == 0), stop=(ci == CK - 1),
                        )
                    nc.vector.tensor_copy(qkT_sb[:, ft, tt * TT:(tt + 1) * TT], ps[:, 0, :])
            # v natural: psum[t128, f256] = xT[c, t].T @ wqkvT[c, v]
            for si in range(NSCH):
                ps = psA.tile([128, 2, TT], F32)
                for ci in range(CK):
                    nc.tensor.matmul(
                        ps[:, 0, 0:FV],
                        xT_sb[:, ci, si * 128:(si + 1) * 128],
                        wqkvT_sb[:, ci, FQK:FQK + FV],
                        start=(ci == 0), stop=(ci == CK - 1),
                    )
                nc.vector.tensor_copy(
                    v_sb[:, si, :, 0:DH],
                    ps[:, 0, 0:FV].rearrange("p (h d) -> p h d", h=NH),
                )

            # ---- attention + normalization + output projection, per t-tile ----
            # S^T orientation: psum[s128, t] = k^T.T @ q^T.  Diagonal s-chunks
            # only need t >= s, so their matmul/exp/PV run at partial width;
            # affine_select zeroes both the causal upper triangle and the
            # stale-psum region the partial matmul skipped.
            for tt in range(NTT):
                # L rows for this t-tile, all on partition 0: [0, pr, hi, :]
                lsq4 = rcp.tile([1, 2, 2, TT], F32, tag="lsq", name=f"lsq{tt}")
                n_ss = 4 * (tt + 1)  # causal: s-chunks 0 .. 4*tt+3
                for pr in range(NPAIR):
                    pv = [psV.tile([DH + 1, TT], F32, tag="pv", name=f"pv{pr}_{tt}_{k}")
                          for k in range(2)]
                    for sq in range(n_ss // 2):
                        diag = sq >= 2 * tt
                        for hi in range(2):
                            h = pr * 2 + hi
                            ps = psA.tile([128, 2, TT], F32)
                            pt = ptp.tile([128, 2, TT], BF16)
                            for i in range(2):
                                ss = 2 * sq + i
                                off = 128 * (ss - 4 * tt) if diag else 0
                                nc.tensor.matmul(
                                    ps[:, i, off:TT],
                                    qkT_sb[hi * 64:(hi + 1) * 64, 2 + pr, ss * 128:(ss + 1) * 128],
                                    qkT_sb[hi * 64:(hi + 1) * 64, pr, tt * TT + off:(tt + 1) * TT],
                                )
                            if diag:
                                for i in range(2):
                                    ss = 2 * sq + i
                                    off = 128 * (ss - 4 * tt)
                                    nc.scalar.activation(
                                        pt[:, i, off:TT], ps[:, i, off:TT], EXP, scale=0.125)
                                    # keep where t >= s:  f - 128*l - p >= 0
                                    nc.gpsimd.affine_select(
                                        out=pt[:, i, :], in_=pt[:, i, :],
                                        compare_op=mybir.AluOpType.is_ge,
                                        fill=0.0,
                                        base=-off,
                                        channel_multiplier=-1,
                                        pattern=[[1, TT]],
                                    )
                            else:
                                nc.scalar.activation(pt, ps, EXP, scale=0.125)
                            for i in range(2):
                                ss = 2 * sq + i
                                off = 128 * (ss - 4 * tt) if diag else 0
                                nc.tensor.matmul(
                                    pv[hi][:, off:TT],
                                    v_sb[:, ss, h, :],
                                    pt[:, i, off:TT],
                                    start=(ss == 0), stop=(ss == n_ss - 1),
                                )
                    for hi in range(2):
                        nc.vector.tensor_copy(
                            oT_sb[hi * 64:(hi + 1) * 64, pr, tt * TT:(tt + 1) * TT],
                            pv[hi][0:DH, :],
                        )
                        nc.vector.tensor_copy(
                            lsq4[0:1, pr, hi, :],
                            pv[hi][DH:DH + 1, :],
                        )

                # softmax normalization for this t-tile: r = 1/L, broadcast
                # r rows across the pair's 128 partitions via sel2 matmul,
                # multiply O^T in place.
                with nc.allow_low_precision("bf16 recip feeds bf16 matmul rhs"):
                    rsq4 = rcp.tile([1, 2, 2, TT], BF16, tag="rsq", name=f"rsq{tt}")
                    nc.vector.reciprocal(rsq4, lsq4)
                for pr in range(NPAIR):
                    bq = psB.tile([128, TT], F32, tag="bq", name=f"bq{pr}_{tt}")
                    for hi in range(2):
                        nc.tensor.matmul(bq, selh[0:1, hi, :], rsq4[0:1, pr, hi, :],
                                         start=(hi == 0), stop=(hi == 1))
                    nc.vector.tensor_mul(
                        oT_sb[:, pr, tt * TT:(tt + 1) * TT],
                        oT_sb[:, pr, tt * TT:(tt + 1) * TT],
                        bq,
                    )

                # output projection for this t-tile's four 128-row chunks
                for tq in range(4 * tt, 4 * (tt + 1)):
                    ps = psA.tile([128, 2, TT], F32, tag="ps", name=f"yq{tq}")
                    for ot in range(2):
                        for pr in range(NPAIR):
                            nc.tensor.matmul(
                                ps[:, ot, :],
                                oT_sb[:, pr, tq * 128:(tq + 1) * 128],
                                woutT_sb[:, pr, ot * TT:(ot + 1) * TT],
                                start=(pr == 0), stop=(pr == NPAIR - 1),
                            )
                    yt = yp.tile([128, C], F32)
                    nc.vector.tensor_copy(yt, ps.rearrange("p a t -> p (a t)"))
                    nc.sync.dma_start(y[tq * 128:(tq + 1) * 128, :], yt)

    nc.compile()
    return nc


_NC_CACHE = None


def _get_nc():
    global _NC_CACHE
    if _NC_CACHE is None:
        _NC_CACHE = build_nc()
    return _NC_CACHE


def make_in_maps(x, W_qkv, W_out):
    bf16 = ml_dtypes.bfloat16
    x = np.asarray(x, dtype=np.float32)
    W_qkv = np.asarray(W_qkv, dtype=np.float32)
    W_out = np.asarray(W_out, dtype=np.float32)
    xT = [np.ascontiguousarray(x[b].T).astype(bf16) for b in range(B)]
    in_maps = []
    for c in range(NCORES):
        b, g = c // 4, c % 4
        rq = W_qkv[g * 256:(g + 1) * 256]            # q rows, heads 4g..4g+3
        rk = W_qkv[C + g * 256:C + (g + 1) * 256]    # k rows
        rv = W_qkv[2 * C + g * 256:2 * C + (g + 1) * 256]  # v rows
        wqkvT = np.ascontiguousarray(
            np.concatenate([rq, rk, rv], axis=0).T).astype(bf16)
        woutT = np.ascontiguousarray(W_out[:, g * 256:(g + 1) * 256].T).astype(bf16)
        in_maps.append({"xT": xT[b], "wqkvT": wqkvT, "woutT": woutT})
    return in_maps


def kernel(x, W_qkv, W_out):
    nc = _get_nc()
    in_maps = make_in_maps(x, W_qkv, W_out)
    res = run_bass_kernel_spmd(nc, in_maps, core_ids=list(range(NCORES)))
    kernel.last_results = res
    y = np.zeros((B, T, C), dtype=np.float32)
    for c in range(NCORES):
        y[c // 4] += res.results[c]["y"]
    return y


# revision 15
# speedup vs baseline: 1.3757x; 1.3757x over previous
"""Causal self-attention (B=2, T=2048, D=1024, H=16) on 8 trn2 cores.

Sharding: tensor-parallel over heads x data-parallel over batch.
Core c handles batch b = c // 4, head group g = c % 4 (heads 4g..4g+3).
Host pre-slices/pre-transposes weight+activation shards (cast to bf16);
each core returns a partial y (its heads' contribution); host sums
groups of 4.

v3: bf16 matmul datapath; causal trimming of diagonal S/PV quads via
partial-width matmuls; software-pipelined schedule: the QKV projection
of t-tile tt+1 and the normalize+project of t-tile tt-1 are emitted
interleaved into the attention loop of t-tile tt, so the PE never
starves while the Scalar engine chews exp (PE idle gaps re-engage the
HAM clock throttle and halve the PE clock).  Softmax normalization
broadcasts L across partitions on GpSimd and takes one wide 128-lane
reciprocal on Vector (a 1-lane reciprocal is ~25x slower).
"""

import os
import sys

for _p in ("/opt/trn_rl_repo", "/root/.axon_site/_ro/trn_rl_repo"):
    if os.path.isdir(_p) and _p not in sys.path:
        sys.path.insert(0, _p)

import ml_dtypes
import numpy as np

import concourse.bass as bass
import concourse.mybir as mybir
import concourse.tile as tile
from concourse import bacc
from concourse.bass_utils import run_bass_kernel_spmd

F32 = mybir.dt.float32
BF16 = mybir.dt.bfloat16
U16 = mybir.dt.uint16

B, T, C = 2, 2048, 1024
NHEAD_TOT = 16
DH = 64
NCORES = 8
NH = 4          # heads per core
NPAIR = 2       # head pairs per core
CK = C // 128   # contraction chunks (8)
TT = 512        # attention t-tile width
NTT = T // TT   # 4
NSCH = T // 128  # s chunks (16)
FQK = 2 * NH * DH  # 512 cols of qkv^T for q+k
FV = NH * DH       # 256 cols for v

ONE_BF16 = 0x3F80


def build_nc(dbg=False):
    nc = bacc.Bacc("TRN2", target_bir_lowering=False, debug=False)

    xT = nc.dram_tensor("xT", [C, T], BF16, kind="ExternalInput")
    wqkvT = nc.dram_tensor("wqkvT", [C, FQK + FV], BF16, kind="ExternalInput")
    woutT = nc.dram_tensor("woutT", [NH * DH, C], BF16, kind="ExternalInput")
    y = nc.dram_tensor("y", [T, C], BF16, kind="ExternalOutput")
    if dbg:
        dbg_qkT = nc.dram_tensor("dbg_qkT", [128, 4, T], BF16, kind="ExternalOutput")
        dbg_v = nc.dram_tensor("dbg_v", [128, NSCH, NH, DH + 1], BF16, kind="ExternalOutput")
        dbg_oT = nc.dram_tensor("dbg_oT", [128, NPAIR, T], BF16, kind="ExternalOutput")
        dbg_L = nc.dram_tensor("dbg_L", [1, NTT, 2, 2, TT], F32, kind="ExternalOutput")

    EXP = mybir.ActivationFunctionType.Exp

    with tile.TileContext(nc) as tc:
        with (
            tc.tile_pool(name="const", bufs=1) as const,
            tc.tile_pool(name="ptp", bufs=6) as ptp,
            tc.tile_pool(name="rcp", bufs=2) as rcp,
            tc.tile_pool(name="yp", bufs=2) as yp,
            tc.tile_pool(name="psA", bufs=3, space="PSUM") as psA,
            tc.tile_pool(name="psV", bufs=2, space="PSUM") as psV,
        ):
            # ---- persistent SBUF ----
            xT_sb = const.tile([128, CK, T], BF16)            # x^T (c-major)
            wqkvT_sb = const.tile([128, CK, FQK + FV], BF16)  # W_qkv^T cols [q|k|v]
            woutT_sb = const.tile([128, NPAIR, C], BF16)      # W_out^T rows per pair
            qkT_sb = const.tile([128, 4, T], BF16)            # [qP0|qP1|kP0|kP1] x T
            v_sb = const.tile([128, NSCH, NH, DH + 1], BF16)  # V (s-major) + ones col
            oT_sb = const.tile([128, NPAIR, T], BF16)         # O^T, pair-stacked

            for ci in range(CK):
                nc.sync.dma_start(xT_sb[:, ci, :], xT[ci * 128:(ci + 1) * 128, :])
                nc.sync.dma_start(wqkvT_sb[:, ci, :], wqkvT[ci * 128:(ci + 1) * 128, :])
            for pr in range(NPAIR):
                nc.sync.dma_start(woutT_sb[:, pr, :], woutT[pr * 128:(pr + 1) * 128, :])
            nc.vector.memset(v_sb[:, :, :, DH:DH + 1].bitcast(U16), ONE_BF16)

            lsq = {}  # tt -> [1, 2, 2, TT] f32 tile of L rows (partition 0)

            def qkv_gen(tt):
                """QKV projection work for t-tile tt: q^T/k^T columns
                tt*TT..(tt+1)*TT and V s-chunks 4tt..4tt+3.  Yields after
                every 2 matmuls so it can be paced into the attention loop."""
                for half in range(2):
                    ps = psA.tile([128, 2, TT], F32, tag="ps", name=f"qkg{tt}_{half}")
                    for ci in range(CK):
                        for j in range(2):
                            ft = half * 2 + j
                            nc.tensor.matmul(
                                ps[:, j, :],
                                wqkvT_sb[:, ci, ft * 128:(ft + 1) * 128],
                                xT_sb[:, ci, tt * TT:(tt + 1) * TT],
                                start=(ci == 0), stop=(ci == CK - 1),
                            )
                        yield
                    for j in range(2):
                        ft = half * 2 + j
                        nc.vector.tensor_copy(qkT_sb[:, ft, tt * TT:(tt + 1) * TT], ps[:, j, :])
                    yield
                for half in range(2):
                    ps = psA.tile([128, 2, TT], F32, tag="ps", name=f"vg{tt}_{half}")
                    for ci in range(CK):
                        for j in range(2):
                            si = 4 * tt + half * 2 + j
                            nc.tensor.matmul(
                                ps[:, j, 0:FV],
                                xT_sb[:, ci, si * 128:(si + 1) * 128],
                                wqkvT_sb[:, ci, FQK:FQK + FV],
                                start=(ci == 0), stop=(ci == CK - 1),
                            )
                        yield
                    for j in range(2):
                        si = 4 * tt + half * 2 + j
                        nc.vector.tensor_copy(
                            v_sb[:, si, :, 0:DH],
                            ps[:, j, 0:FV].rearrange("p (h d) -> p h d", h=NH),
                        )
                    yield

            def norm_proj_gen(tt):
                """Softmax-normalize O^T of t-tile tt, then project to y."""
                for pr in range(NPAIR):
                    # both hi rows of L replicated to all 128 partitions
                    Lb = rcp.tile([128, 2, TT], F32, tag="Lb", name=f"Lb{pr}_{tt}")
                    nc.gpsimd.partition_broadcast(Lb, lsq[tt][0:1, pr, :, :],
                                                  channels=128)
                    yield
                    rq = rcp.tile([128, 2, TT], BF16, tag="rq", name=f"rq{pr}_{tt}")
                    with nc.allow_low_precision("1/L feeds bf16 normalize mul"):
                        nc.vector.reciprocal(rq, Lb)
                    yield
                    for hi in range(2):
                        nc.vector.tensor_mul(
                            oT_sb[hi * 64:(hi + 1) * 64, pr, tt * TT:(tt + 1) * TT],
                            oT_sb[hi * 64:(hi + 1) * 64, pr, tt * TT:(tt + 1) * TT],
                            rq[hi * 64:(hi + 1) * 64, hi, :],
                        )
                    yield
                for tq in range(4 * tt, 4 * (tt + 1)):
                    ps = psA.tile([128, 2, TT], F32, tag="ps", name=f"yq{tq}")
                    for ot in range(2):
                        for pr in range(NPAIR):
                            nc.tensor.matmul(
                                ps[:, ot, :],
                                oT_sb[:, pr, tq * 128:(tq + 1) * 128],
                                woutT_sb[:, pr, ot * TT:(ot + 1) * TT],
                                start=(pr == 0), stop=(pr == NPAIR - 1),
                            )
                        yield
                    yt = yp.tile([128, C], BF16)
                    nc.vector.tensor_copy(yt, ps.rearrange("p a t -> p (a t)"))
                    nc.sync.dma_start(y[tq * 128:(tq + 1) * 128, :], yt)
                    yield

            def pull(feeders, k):
                done = 0
                while done < k and feeders:
                    try:
                        next(feeders[0])
                        done += 1
                    except StopIteration:
                        feeders.pop(0)

            # prologue: QKV for t-tile 0 runs un-paced
            pull([qkv_gen(0)], 10 ** 9)

            # ---- attention (S^T orientation), pipelined across t-tiles ----
            # Diagonal s-chunks only need t >= s: their S matmul, exp and PV
            # run at partial width; affine_select zeroes both the causal
            # upper triangle and the stale-psum region the partial matmul
            # skipped.
            for tt in range(NTT):
                feeders = []
                if tt + 1 < NTT:
                    feeders.append(qkv_gen(tt + 1))
                if tt > 0:
                    feeders.append(norm_proj_gen(tt - 1))
                n_ss = 4 * (tt + 1)  # causal: s-chunks 0 .. 4*tt+3
                iters = 2 * (n_ss // 2) * NPAIR
                budget = 36 + (18 if tt > 0 else 0)
                per = -(-budget // iters)  # ceil
                lsq[tt] = rcp.tile([1, 2, 2, TT], F32, tag="lsq", name=f"lsq{tt}")
                for pr in range(NPAIR):
                    pv = [psV.tile([DH + 1, TT], F32, tag="pv", name=f"pv{pr}_{tt}_{k}")
                          for k in range(2)]
                    for sq in range(n_ss // 2):
                        diag = sq >= 2 * tt
                        for hi in range(2):
                            h = pr * 2 + hi
                            ps = psA.tile([128, 2, TT], F32, tag="ps")
                            pt = ptp.tile([128, 2, TT], BF16)
                            for i in range(2):
                                ss = 2 * sq + i
                                off = 128 * (ss - 4 * tt) if diag else 0
                                nc.tensor.matmul(
                                    ps[:, i, off:TT],
                                    qkT_sb[hi * 64:(hi + 1) * 64, 2 + pr, ss * 128:(ss + 1) * 128],
                                    qkT_sb[hi * 64:(hi + 1) * 64, pr, tt * TT + off:(tt + 1) * TT],
                                )
                            if diag:
                                for i in range(2):
                                    ss = 2 * sq + i
                                    off = 128 * (ss - 4 * tt)
                                    nc.scalar.activation(
                                        pt[:, i, off:TT], ps[:, i, off:TT], EXP, scale=0.125)
                                    # keep where t >= s:  f - 128*l - p >= 0
                                    nc.gpsimd.affine_select(
                                        out=pt[:, i, :], in_=pt[:, i, :],
                                        compare_op=mybir.AluOpType.is_ge,
                                        fill=0.0,
                                        base=-off,
                                        channel_multiplier=-1,
                                        pattern=[[1, TT]],
                                    )
                            else:
                                nc.scalar.activation(pt, ps, EXP, scale=0.125)
                            for i in range(2):
                                ss = 2 * sq + i
                                off = 128 * (ss - 4 * tt) if diag else 0
                                nc.tensor.matmul(
                                    pv[hi][:, off:TT],
                                    v_sb[:, ss, h, :],
                                    pt[:, i, off:TT],
                                    start=(ss == 0), stop=(ss == n_ss - 1),
                                )
                            pull(feeders, per)
                    for hi in range(2):
                        nc.vector.tensor_copy(
                            oT_sb[hi * 64:(hi + 1) * 64, pr, tt * TT:(tt + 1) * TT],
                            pv[hi][0:DH, :],
                        )
                        nc.vector.tensor_copy(
                            lsq[tt][0:1, pr, hi, :],
                            pv[hi][DH:DH + 1, :],
                        )
                pull(feeders, 10 ** 9)

            if dbg:
                for tt in range(NTT):
                    nc.sync.dma_start(dbg_L[0:1, tt], lsq[tt][0:1])
                nc.sync.dma_start(dbg_oT[:], oT_sb)

            # epilogue: normalize + project the last t-tile
            pull([norm_proj_gen(NTT - 1)], 10 ** 9)

            if dbg:
                nc.sync.dma_start(dbg_qkT[:], qkT_sb)
                nc.sync.dma_start(dbg_v[:], v_sb)

    nc.compile()
    return nc


_NC_CACHE = None


def _get_nc():
    global _NC_CACHE
    if _NC_CACHE is None:
        _NC_CACHE = build_nc()
    return _NC_CACHE


def make_in_maps(x, W_qkv, W_out):
    bf16 = ml_dtypes.bfloat16
    x = np.asarray(x, dtype=np.float32)
    W_qkv = np.asarray(W_qkv, dtype=np.float32)
    W_out = np.asarray(W_out, dtype=np.float32)
    xT = [np.ascontiguousarray(x[b].T).astype(bf16) for b in range(B)]
    in_maps = []
    for c in range(NCORES):
        b, g = c // 4, c % 4
        rq = W_qkv[g * 256:(g + 1) * 256]            # q rows, heads 4g..4g+3
        rk = W_qkv[C + g * 256:C + (g + 1) * 256]    # k rows
        rv = W_qkv[2 * C + g * 256:2 * C + (g + 1) * 256]  # v rows
        wqkvT = np.ascontiguousarray(
            np.concatenate([rq, rk, rv], axis=0).T).astype(bf16)
        woutT = np.ascontiguousarray(W_out[:, g * 256:(g + 1) * 256].T).astype(bf16)
        in_maps.append({"xT": xT[b], "wqkvT": wqkvT, "woutT": woutT})
    return in_maps


def kernel(x, W_qkv, W_out):
    nc = _get_nc()
    in_maps = make_in_maps(x, W_qkv, W_out)
    res = run_bass_kernel_spmd(nc, in_maps, core_ids=list(range(NCORES)))
    kernel.last_results = res
    y = np.zeros((B, T, C), dtype=np.float32)
    for c in range(NCORES):
        y[c // 4] += res.results[c]["y"].astype(np.float32)
    return y


# revision 19
# speedup vs baseline: 1.5100x; 1.0976x over previous
"""Causal self-attention (B=2, T=2048, D=1024, H=16) on 8 trn2 cores.

Sharding: tensor-parallel over heads x data-parallel over batch.
Core c handles batch b = c // 4, head group g = c % 4 (heads 4g..4g+3).
Host pre-slices/pre-transposes weight+activation shards (cast to bf16);
each core returns a partial y (its heads' contribution); host sums
groups of 4.

v3: bf16 matmul datapath; causal trimming of diagonal S/PV quads via
partial-width matmuls; software-pipelined schedule: the QKV projection
of t-tile tt+1 and the normalize+project of t-tile tt-1 are emitted
interleaved into the attention loop of t-tile tt, so the PE never
starves while the Scalar engine chews exp (PE idle gaps re-engage the
HAM clock throttle and halve the PE clock).  Softmax normalization
broadcasts L across partitions on GpSimd and takes one wide 128-lane
reciprocal on Vector (a 1-lane reciprocal is ~25x slower).
"""

import os
import sys

for _p in ("/opt/trn_rl_repo", "/root/.axon_site/_ro/trn_rl_repo"):
    if os.path.isdir(_p) and _p not in sys.path:
        sys.path.insert(0, _p)

import ml_dtypes
import numpy as np

import concourse.bass as bass
import concourse.mybir as mybir
import concourse.tile as tile
from concourse import bacc
from concourse.bass_utils import run_bass_kernel_spmd

F32 = mybir.dt.float32
BF16 = mybir.dt.bfloat16
U16 = mybir.dt.uint16

B, T, C = 2, 2048, 1024
NHEAD_TOT = 16
DH = 64
NCORES = 8
NH = 4          # heads per core
NPAIR = 2       # head pairs per core
CK = C // 128   # contraction chunks (8)
TT = 512        # attention t-tile width
NTT = T // TT   # 4
NSCH = T // 128  # s chunks (16)
FQK = 2 * NH * DH  # 512 cols of qkv^T for q+k
FV = NH * DH       # 256 cols for v

ONE_BF16 = 0x3F80


def build_nc(dbg=False):
    nc = bacc.Bacc("TRN2", target_bir_lowering=False, debug=False)

    xT = nc.dram_tensor("xT", [C, T], BF16, kind="ExternalInput")
    wqkvT = nc.dram_tensor("wqkvT", [C, FQK + FV], BF16, kind="ExternalInput")
    woutT = nc.dram_tensor("woutT", [NH * DH, C], BF16, kind="ExternalInput")
    y = nc.dram_tensor("y", [T, C], BF16, kind="ExternalOutput")
    if dbg:
        dbg_qkT = nc.dram_tensor("dbg_qkT", [128, 4, T], BF16, kind="ExternalOutput")
        dbg_v = nc.dram_tensor("dbg_v", [128, NSCH, NH, DH + 1], BF16, kind="ExternalOutput")
        dbg_oT = nc.dram_tensor("dbg_oT", [128, NPAIR, T], BF16, kind="ExternalOutput")
        dbg_L = nc.dram_tensor("dbg_L", [1, NTT, 2, 2, TT], F32, kind="ExternalOutput")

    EXP = mybir.ActivationFunctionType.Exp

    with tile.TileContext(nc) as tc:
        with (
            tc.tile_pool(name="const", bufs=1) as const,
            tc.tile_pool(name="ptp", bufs=6) as ptp,
            tc.tile_pool(name="rcp", bufs=2) as rcp,
            tc.tile_pool(name="yp", bufs=2) as yp,
            tc.tile_pool(name="psA", bufs=3, space="PSUM") as psA,
            tc.tile_pool(name="psV", bufs=2, space="PSUM") as psV,
        ):
            # ---- persistent SBUF ----
            xT_sb = const.tile([128, CK, T], BF16)            # x^T (c-major)
            wqkvT_sb = const.tile([128, CK, FQK + FV], BF16)  # W_qkv^T cols [q|k|v]
            woutT_sb = const.tile([128, NPAIR, C], BF16)      # W_out^T rows per pair
            qkT_sb = const.tile([128, 4, T], BF16)            # [qP0|qP1|kP0|kP1] x T
            v_sb = const.tile([128, NSCH, NH, DH + 1], BF16)  # V (s-major) + ones col
            oT_sb = const.tile([128, NPAIR, T], BF16)         # O^T, pair-stacked

            for ci in range(CK):
                nc.sync.dma_start(xT_sb[:, ci, :], xT[ci * 128:(ci + 1) * 128, :])
                nc.sync.dma_start(wqkvT_sb[:, ci, :], wqkvT[ci * 128:(ci + 1) * 128, :])
            for pr in range(NPAIR):
                nc.sync.dma_start(woutT_sb[:, pr, :], woutT[pr * 128:(pr + 1) * 128, :])
            nc.vector.memset(v_sb[:, :, :, DH:DH + 1].bitcast(U16), ONE_BF16)

            lsq = {}  # tt -> [1, 2, 2, TT] f32 tile of L rows (partition 0)

            def qkv_gen(tt):
                """QKV projection work for t-tile tt: q^T/k^T columns
                tt*TT..(tt+1)*TT and V s-chunks 4tt..4tt+3.  Yields after
                every 2 matmuls so it can be paced into the attention loop."""
                for half in range(2):
                    ps = psA.tile([128, 2, TT], F32, tag="ps", name=f"qkg{tt}_{half}")
                    for ci in range(CK):
                        for j in range(2):
                            ft = half * 2 + j
                            nc.tensor.matmul(
                                ps[:, j, :],
                                wqkvT_sb[:, ci, ft * 128:(ft + 1) * 128],
                                xT_sb[:, ci, tt * TT:(tt + 1) * TT],
                                start=(ci == 0), stop=(ci == CK - 1),
                            )
                        yield
                    for j in range(2):
                        ft = half * 2 + j
                        nc.vector.tensor_copy(qkT_sb[:, ft, tt * TT:(tt + 1) * TT], ps[:, j, :])
                    yield
                for half in range(2):
                    ps = psA.tile([128, 2, TT], F32, tag="ps", name=f"vg{tt}_{half}")
                    for ci in range(CK):
                        for j in range(2):
                            si = 4 * tt + half * 2 + j
                            nc.tensor.matmul(
                                ps[:, j, 0:FV],
                                xT_sb[:, ci, si * 128:(si + 1) * 128],
                                wqkvT_sb[:, ci, FQK:FQK + FV],
                                start=(ci == 0), stop=(ci == CK - 1),
                            )
                        yield
                    for j in range(2):
                        si = 4 * tt + half * 2 + j
                        nc.vector.tensor_copy(
                            v_sb[:, si, :, 0:DH],
                            ps[:, j, 0:FV].rearrange("p (h d) -> p h d", h=NH),
                        )
                    yield

            def norm_emit(tt, pr):
                """Softmax-normalize O^T of (tt, pr): broadcast both hi rows
                of L to all partitions (GpSimd), one fast 128-lane reciprocal,
                two in-place multiplies."""
                Lb = rcp.tile([128, 2, TT], F32, tag="Lb", name=f"Lb{pr}_{tt}")
                nc.gpsimd.partition_broadcast(Lb, lsq[tt][0:1, pr, :, :],
                                              channels=128)
                rq = rcp.tile([128, 2, TT], F32, tag="rq", name=f"rq{pr}_{tt}")
                nc.vector.reciprocal_approx_fast(rq, Lb)
                for hi in range(2):
                    nc.vector.tensor_mul(
                        oT_sb[hi * 64:(hi + 1) * 64, pr, tt * TT:(tt + 1) * TT],
                        oT_sb[hi * 64:(hi + 1) * 64, pr, tt * TT:(tt + 1) * TT],
                        rq[hi * 64:(hi + 1) * 64, hi, :],
                    )

            def proj_gen(tt):
                """Project normalized O^T of t-tile tt to y."""
                for tq in range(4 * tt, 4 * (tt + 1)):
                    ps = psA.tile([128, 2, TT], F32, tag="ps", name=f"yq{tq}")
                    for ot in range(2):
                        for pr in range(NPAIR):
                            nc.tensor.matmul(
                                ps[:, ot, :],
                                oT_sb[:, pr, tq * 128:(tq + 1) * 128],
                                woutT_sb[:, pr, ot * TT:(ot + 1) * TT],
                                start=(pr == 0), stop=(pr == NPAIR - 1),
                            )
                        yield
                    yt = yp.tile([128, C], BF16)
                    nc.vector.tensor_copy(yt, ps.rearrange("p a t -> p (a t)"))
                    nc.sync.dma_start(y[tq * 128:(tq + 1) * 128, :], yt)
                    yield

            def pull(feeders, k):
                done = 0
                while done < k and feeders:
                    try:
                        next(feeders[0])
                        done += 1
                    except StopIteration:
                        feeders.pop(0)

            # prologue: QKV for t-tile 0 runs un-paced
            pull([qkv_gen(0)], 10 ** 9)

            # ---- attention (S^T orientation), pipelined across t-tiles ----
            # Diagonal s-chunks only need t >= s: their S matmul, exp and PV
            # run at partial width; affine_select zeroes both the causal
            # upper triangle and the stale-psum region the partial matmul
            # skipped.
            for tt in range(NTT):
                feeders = []
                if tt > 0:
                    feeders.append(proj_gen(tt - 1))
                if tt + 1 < NTT:
                    feeders.append(qkv_gen(tt + 1))
                n_ss = 4 * (tt + 1)  # causal: s-chunks 0 .. 4*tt+3
                iters = 2 * (n_ss // 2) * NPAIR
                budget = 36 + (20 if tt > 0 else 0)
                per = -(-budget // iters)  # ceil
                lsq[tt] = rcp.tile([1, 2, 2, TT], F32, tag="lsq", name=f"lsq{tt}")
                for pr in range(NPAIR):
                    pv = [psV.tile([DH + 1, TT], F32, tag="pv", name=f"pv{pr}_{tt}_{k}")
                          for k in range(2)]
                    for sq in range(n_ss // 2):
                        diag = sq >= 2 * tt
                        for hi in range(2):
                            h = pr * 2 + hi
                            ps = psA.tile([128, 2, TT], F32, tag="ps")
                            pt = ptp.tile([128, 2, TT], BF16)
                            for i in range(2):
                                ss = 2 * sq + i
                                off = 128 * (ss - 4 * tt) if diag else 0
                                nc.tensor.matmul(
                                    ps[:, i, off:TT],
                                    qkT_sb[hi * 64:(hi + 1) * 64, 2 + pr, ss * 128:(ss + 1) * 128],
                                    qkT_sb[hi * 64:(hi + 1) * 64, pr, tt * TT + off:(tt + 1) * TT],
                                )
                            if diag:
                                for i in range(2):
                                    ss = 2 * sq + i
                                    off = 128 * (ss - 4 * tt)
                                    nc.scalar.activation(
                                        pt[:, i, off:TT], ps[:, i, off:TT], EXP, scale=0.125)
                                    # keep where t >= s:  f - 128*l - p >= 0
                                    nc.gpsimd.affine_select(
                                        out=pt[:, i, :], in_=pt[:, i, :],
                                        compare_op=mybir.AluOpType.is_ge,
                                        fill=0.0,
                                        base=-off,
                                        channel_multiplier=-1,
                                        pattern=[[1, TT]],
                                    )
                            else:
                                nc.scalar.activation(pt, ps, EXP, scale=0.125)
                            for i in range(2):
                                ss = 2 * sq + i
                                off = 128 * (ss - 4 * tt) if diag else 0
                                nc.tensor.matmul(
                                    pv[hi][:, off:TT],
                                    v_sb[:, ss, h, :],
                                    pt[:, i, off:TT],
                                    start=(ss == 0), stop=(ss == n_ss - 1),
                                )
                            pull(feeders, per)
                    for hi in range(2):
                        nc.vector.tensor_copy(
                            oT_sb[hi * 64:(hi + 1) * 64, pr, tt * TT:(tt + 1) * TT],
                            pv[hi][0:DH, :],
                        )
                        nc.vector.tensor_copy(
                            lsq[tt][0:1, pr, hi, :],
                            pv[hi][DH:DH + 1, :],
                        )
                    norm_emit(tt, pr)
                pull(feeders, 10 ** 9)

            if dbg:
                for tt in range(NTT):
                    nc.sync.dma_start(dbg_L[0:1, tt], lsq[tt][0:1])
                nc.sync.dma_start(dbg_oT[:], oT_sb)

            # epilogue: project the last t-tile
            pull([proj_gen(NTT - 1)], 10 ** 9)

            if dbg:
                nc.sync.dma_start(dbg_qkT[:], qkT_sb)
                nc.sync.dma_start(dbg_v[:], v_sb)

    nc.compile()
    return nc


_NC_CACHE = None


def _get_nc():
    global _NC_CACHE
    if _NC_CACHE is None:
        _NC_CACHE = build_nc()
    return _NC_CACHE


def make_in_maps(x, W_qkv, W_out):
    bf16 = ml_dtypes.bfloat16
    x = np.asarray(x, dtype=np.float32)
    W_qkv = np.asarray(W_qkv, dtype=np.float32)
    W_out = np.asarray(W_out, dtype=np.float32)
    xT = [np.ascontiguousarray(x[b].T).astype(bf16) for b in range(B)]
    in_maps = []
    for c in range(NCORES):
        b, g = c // 4, c % 4
        rq = W_qkv[g * 256:(g + 1) * 256]            # q rows, heads 4g..4g+3
        rk = W_qkv[C + g * 256:C + (g + 1) * 256]    # k rows
        rv = W_qkv[2 * C + g * 256:2 * C + (g + 1) * 256]  # v rows
        wqkvT = np.ascontiguousarray(
            np.concatenate([rq, rk, rv], axis=0).T).astype(bf16)
        woutT = np.ascontiguousarray(W_out[:, g * 256:(g + 1) * 256].T).astype(bf16)
        in_maps.append({"xT": xT[b], "wqkvT": wqkvT, "woutT": woutT})
    return in_maps


def kernel(x, W_qkv, W_out):
    nc = _get_nc()
    in_maps = make_in_maps(x, W_qkv, W_out)
    res = run_bass_kernel_spmd(nc, in_maps, core_ids=list(range(NCORES)))
    kernel.last_results = res
    y = np.zeros((B, T, C), dtype=np.float32)
    for c in range(NCORES):
        y[c // 4] += res.results[c]["y"].astype(np.float32)
    return y


# revision 22
# speedup vs baseline: 1.6662x; 1.1034x over previous
"""Causal self-attention (B=2, T=2048, D=1024, H=16) on 8 trn2 cores.

Sharding: tensor-parallel over heads x data-parallel over batch.
Core c handles batch b = c // 4, head group g = c % 4 (heads 4g..4g+3).
Host pre-slices/pre-transposes weight+activation shards (cast to bf16);
each core returns a partial y (its heads' contribution); host sums
groups of 4.

v3: bf16 matmul datapath; causal trimming of diagonal S/PV quads via
partial-width matmuls; software-pipelined schedule: the QKV projection
of t-tile tt+1 and the normalize+project of t-tile tt-1 are emitted
interleaved into the attention loop of t-tile tt, so the PE never
starves while the Scalar engine chews exp (PE idle gaps re-engage the
HAM clock throttle and halve the PE clock).  Softmax normalization
broadcasts L across partitions on GpSimd and takes one wide 128-lane
reciprocal on Vector (a 1-lane reciprocal is ~25x slower).
"""

import os
import sys

for _p in ("/opt/trn_rl_repo", "/root/.axon_site/_ro/trn_rl_repo"):
    if os.path.isdir(_p) and _p not in sys.path:
        sys.path.insert(0, _p)

import ml_dtypes
import numpy as np

import concourse.bass as bass
import concourse.mybir as mybir
import concourse.tile as tile
from concourse import bacc
from concourse.bass_utils import run_bass_kernel_spmd

F32 = mybir.dt.float32
BF16 = mybir.dt.bfloat16
U16 = mybir.dt.uint16

B, T, C = 2, 2048, 1024
NHEAD_TOT = 16
DH = 64
NCORES = 8
NH = 4          # heads per core
NPAIR = 2       # head pairs per core
CK = C // 128   # contraction chunks (8)
TT = 512        # attention t-tile width
NTT = T // TT   # 4
NSCH = T // 128  # s chunks (16)
FQK = 2 * NH * DH  # 512 cols of qkv^T for q+k
FV = NH * DH       # 256 cols for v

ONE_BF16 = 0x3F80


def build_nc(dbg=False):
    nc = bacc.Bacc("TRN2", target_bir_lowering=False, debug=False)

    xT = nc.dram_tensor("xT", [C, T], BF16, kind="ExternalInput")
    wqkvT = nc.dram_tensor("wqkvT", [C, FQK + FV], BF16, kind="ExternalInput")
    woutT = nc.dram_tensor("woutT", [NH * DH, C], BF16, kind="ExternalInput")
    y = nc.dram_tensor("y", [T, C], BF16, kind="ExternalOutput")
    if dbg:
        dbg_qkT = nc.dram_tensor("dbg_qkT", [128, 4, T], BF16, kind="ExternalOutput")
        dbg_v = nc.dram_tensor("dbg_v", [128, NSCH, NH, DH + 1], BF16, kind="ExternalOutput")
        dbg_oT = nc.dram_tensor("dbg_oT", [128, NPAIR, T], BF16, kind="ExternalOutput")
        dbg_L = nc.dram_tensor("dbg_L", [1, NTT, 2, 2, TT], F32, kind="ExternalOutput")

    EXP = mybir.ActivationFunctionType.Exp

    with tile.TileContext(nc) as tc:
        with (
            tc.tile_pool(name="const", bufs=1) as const,
            tc.tile_pool(name="ptp", bufs=6) as ptp,
            tc.tile_pool(name="rcp", bufs=2) as rcp,
            tc.tile_pool(name="yp", bufs=2) as yp,
            tc.tile_pool(name="psA", bufs=3, space="PSUM") as psA,
            tc.tile_pool(name="psV", bufs=2, space="PSUM") as psV,
        ):
            # ---- persistent SBUF ----
            xT_sb = const.tile([128, CK, T], BF16)            # x^T (c-major)
            wqkvT_sb = const.tile([128, CK, FQK + FV], BF16)  # W_qkv^T cols [q|k|v]
            woutT_sb = const.tile([128, NPAIR, C], BF16)      # W_out^T rows per pair
            qkT_sb = const.tile([128, 4, T], BF16)            # [qP0|qP1|kP0|kP1] x T
            v_sb = const.tile([128, NSCH, NH, DH + 1], BF16)  # V (s-major) + ones col
            oT_sb = const.tile([128, NPAIR, T], BF16)         # O^T, pair-stacked

            for ci in range(CK):
                nc.sync.dma_start(xT_sb[:, ci, :], xT[ci * 128:(ci + 1) * 128, :])
                nc.sync.dma_start(wqkvT_sb[:, ci, :], wqkvT[ci * 128:(ci + 1) * 128, :])
            for pr in range(NPAIR):
                nc.sync.dma_start(woutT_sb[:, pr, :], woutT[pr * 128:(pr + 1) * 128, :])
            nc.vector.memset(v_sb[:, :, :, DH:DH + 1].bitcast(U16), ONE_BF16)

            lsq = {}  # tt -> [1, 2, 2, TT] f32 tile of L rows (partition 0)

            def qkv_gen(tt):
                """QKV projection work for t-tile tt: q^T/k^T columns
                tt*TT..(tt+1)*TT and V s-chunks 4tt..4tt+3.  Yields after
                every 2 matmuls so it can be paced into the attention loop."""
                for half in range(2):
                    ps = psA.tile([128, 2, TT], F32, tag="ps", name=f"qkg{tt}_{half}")
                    for ci in range(CK):
                        for j in range(2):
                            ft = half * 2 + j
                            nc.tensor.matmul(
                                ps[:, j, :],
                                wqkvT_sb[:, ci, ft * 128:(ft + 1) * 128],
                                xT_sb[:, ci, tt * TT:(tt + 1) * TT],
                                start=(ci == 0), stop=(ci == CK - 1),
                            )
                        yield
                    for j in range(2):
                        ft = half * 2 + j
                        nc.vector.tensor_copy(qkT_sb[:, ft, tt * TT:(tt + 1) * TT], ps[:, j, :])
                    yield
                for half in range(2):
                    ps = psA.tile([128, 2, TT], F32, tag="ps", name=f"vg{tt}_{half}")
                    for ci in range(CK):
                        for j in range(2):
                            si = 4 * tt + half * 2 + j
                            nc.tensor.matmul(
                                ps[:, j, 0:FV],
                                xT_sb[:, ci, si * 128:(si + 1) * 128],
                                wqkvT_sb[:, ci, FQK:FQK + FV],
                                start=(ci == 0), stop=(ci == CK - 1),
                            )
                        yield
                    for j in range(2):
                        si = 4 * tt + half * 2 + j
                        nc.vector.tensor_copy(
                            v_sb[:, si, :, 0:DH],
                            ps[:, j, 0:FV].rearrange("p (h d) -> p h d", h=NH),
                        )
                    yield

            def norm_emit(tt, pr):
                """Softmax-normalize O^T of (tt, pr): broadcast both hi rows
                of L to all partitions (GpSimd), one fast 128-lane reciprocal,
                two in-place multiplies."""
                Lb = rcp.tile([128, 2, TT], F32, tag="Lb", name=f"Lb{pr}_{tt}")
                nc.gpsimd.partition_broadcast(Lb, lsq[tt][0:1, pr, :, :],
                                              channels=128)
                rq = rcp.tile([128, 2, TT], F32, tag="rq", name=f"rq{pr}_{tt}")
                nc.vector.reciprocal_approx_fast(rq, Lb)
                for hi in range(2):
                    nc.vector.tensor_mul(
                        oT_sb[hi * 64:(hi + 1) * 64, pr, tt * TT:(tt + 1) * TT],
                        oT_sb[hi * 64:(hi + 1) * 64, pr, tt * TT:(tt + 1) * TT],
                        rq[hi * 64:(hi + 1) * 64, hi, :],
                    )

            def proj_gen(tt):
                """Project normalized O^T of t-tile tt to y."""
                for tq in range(4 * tt, 4 * (tt + 1)):
                    ps = psA.tile([128, 2, TT], F32, tag="ps", name=f"yq{tq}")
                    for ot in range(2):
                        for pr in range(NPAIR):
                            nc.tensor.matmul(
                                ps[:, ot, :],
                                oT_sb[:, pr, tq * 128:(tq + 1) * 128],
                                woutT_sb[:, pr, ot * TT:(ot + 1) * TT],
                                start=(pr == 0), stop=(pr == NPAIR - 1),
                            )
                        yield
                    yt = yp.tile([128, C], BF16)
                    nc.vector.tensor_copy(yt, ps.rearrange("p a t -> p (a t)"))
                    nc.sync.dma_start(y[tq * 128:(tq + 1) * 128, :], yt)
                    yield

            def pull(feeders, k):
                done = 0
                while done < k and feeders:
                    try:
                        next(feeders[0])
                        done += 1
                    except StopIteration:
                        feeders.pop(0)

            # prologue: QKV for t-tile 0 runs un-paced
            pull([qkv_gen(0)], 10 ** 9)

            # ---- attention (S^T orientation), pipelined across t-tiles ----
            # Diagonal s-chunks only need t >= s: their S matmul, exp and PV
            # run at partial width; affine_select zeroes both the causal
            # upper triangle and the stale-psum region the partial matmul
            # skipped.
            for tt in range(NTT):
                # qkv first: its matmuls are dependency-free, while proj
                # waits on the just-emitted norm chain (head-of-line risk)
                feeders = []
                if tt + 1 < NTT:
                    feeders.append(qkv_gen(tt + 1))
                if tt > 0:
                    feeders.append(proj_gen(tt - 1))
                n_ss = 4 * (tt + 1)  # causal: s-chunks 0 .. 4*tt+3
                iters = 2 * (n_ss // 2) * NPAIR
                budget = 36 + (20 if tt > 0 else 0)
                per = -(-budget // iters)  # ceil
                lsq[tt] = rcp.tile([1, 2, 2, TT], F32, tag="lsq", name=f"lsq{tt}")
                for pr in range(NPAIR):
                    pv = [psV.tile([DH + 1, TT], F32, tag="pv", name=f"pv{pr}_{tt}_{k}")
                          for k in range(2)]
                    for sq in range(n_ss // 2):
                        diag = sq >= 2 * tt
                        for hi in range(2):
                            h = pr * 2 + hi
                            ps = psA.tile([128, 2, TT], F32, tag="ps")
                            pt = ptp.tile([128, 2, TT], BF16)
                            for i in range(2):
                                ss = 2 * sq + i
                                off = 128 * (ss - 4 * tt) if diag else 0
                                nc.tensor.matmul(
                                    ps[:, i, off:TT],
                                    qkT_sb[hi * 64:(hi + 1) * 64, 2 + pr, ss * 128:(ss + 1) * 128],
                                    qkT_sb[hi * 64:(hi + 1) * 64, pr, tt * TT + off:(tt + 1) * TT],
                                )
                            if diag:
                                # one strided exp covering both chunks from the
                                # first chunk's causal offset; the selects zero
                                # the masked + stale-psum region afterwards
                                off0 = 128 * (2 * sq - 4 * tt)
                                nc.scalar.activation(
                                    pt[:, :, off0:TT], ps[:, :, off0:TT], EXP, scale=0.125)
                                for i in range(2):
                                    ss = 2 * sq + i
                                    off = 128 * (ss - 4 * tt)
                                    # keep where t >= s:  f - 128*l - p >= 0
                                    nc.gpsimd.affine_select(
                                        out=pt[:, i, :], in_=pt[:, i, :],
                                        compare_op=mybir.AluOpType.is_ge,
                                        fill=0.0,
                                        base=-off,
                                        channel_multiplier=-1,
                                        pattern=[[1, TT]],
                                    )
                            else:
                                nc.scalar.activation(pt, ps, EXP, scale=0.125)
                            for i in range(2):
                                ss = 2 * sq + i
                                off = 128 * (ss - 4 * tt) if diag else 0
                                nc.tensor.matmul(
                                    pv[hi][:, off:TT],
                                    v_sb[:, ss, h, :],
                                    pt[:, i, off:TT],
                                    start=(ss == 0), stop=(ss == n_ss - 1),
                                )
                            pull(feeders, per)
                    for hi in range(2):  # L rows first: they gate the norm chain
                        nc.vector.tensor_copy(
                            lsq[tt][0:1, pr, hi, :],
                            pv[hi][DH:DH + 1, :],
                        )
                    for hi in range(2):
                        nc.vector.tensor_copy(
                            oT_sb[hi * 64:(hi + 1) * 64, pr, tt * TT:(tt + 1) * TT],
                            pv[hi][0:DH, :],
                        )
                    norm_emit(tt, pr)
                pull(feeders, 10 ** 9)

            if dbg:
                for tt in range(NTT):
                    nc.sync.dma_start(dbg_L[0:1, tt], lsq[tt][0:1])
                nc.sync.dma_start(dbg_oT[:], oT_sb)

            # epilogue: project the last t-tile
            pull([proj_gen(NTT - 1)], 10 ** 9)

            if dbg:
                nc.sync.dma_start(dbg_qkT[:], qkT_sb)
                nc.sync.dma_start(dbg_v[:], v_sb)

    nc.compile()
    return nc


_NC_CACHE = None


def _get_nc():
    global _NC_CACHE
    if _NC_CACHE is None:
        _NC_CACHE = build_nc()
    return _NC_CACHE


def make_in_maps(x, W_qkv, W_out):
    bf16 = ml_dtypes.bfloat16
    x = np.asarray(x, dtype=np.float32)
    W_qkv = np.asarray(W_qkv, dtype=np.float32)
    W_out = np.asarray(W_out, dtype=np.float32)
    xT = [np.ascontiguousarray(x[b].T).astype(bf16) for b in range(B)]
    in_maps = []
    for c in range(NCORES):
        b, g = c // 4, c % 4
        rq = W_qkv[g * 256:(g + 1) * 256]            # q rows, heads 4g..4g+3
        rk = W_qkv[C + g * 256:C + (g + 1) * 256]    # k rows
        rv = W_qkv[2 * C + g * 256:2 * C + (g + 1) * 256]  # v rows
        wqkvT = np.ascontiguousarray(
            np.concatenate([rq, rk, rv], axis=0).T).astype(bf16)
        woutT = np.ascontiguousarray(W_out[:, g * 256:(g + 1) * 256].T).astype(bf16)
        in_maps.append({"xT": xT[b], "wqkvT": wqkvT, "woutT": woutT})
    return in_maps


def kernel(x, W_qkv, W_out):
    nc = _get_nc()
    in_maps = make_in_maps(x, W_qkv, W_out)
    res = run_bass_kernel_spmd(nc, in_maps, core_ids=list(range(NCORES)))
    kernel.last_results = res
    y = np.zeros((B, T, C), dtype=np.float32)
    for c in range(NCORES):
        y[c // 4] += res.results[c]["y"].astype(np.float32)
    return y
